# revision 14
# baseline (speedup 1.0000x reference)
"""MoE ExpertGroup kernel for Trainium2 (8 NeuronCores, expert-parallel).

Problem: E=8 experts, H=1024, I=4096, N=16384 tokens sorted by expert.
y[t] = gelu_tanh(x[t] @ w1[e(t)]) @ w2[e(t)]

Sharding: expert-parallel - core e holds expert e's weights and processes
expert e's contiguous token block (balanced routing: 2048 tokens/core).

All matmul operands are bf16 (PE streams 512-row matmuls at ~216 ns vs
227 ns for fp32r); PSUM accumulation and the final y stay fp32. Weights
are packed host-side into partition-major layouts so each logical load
is one contiguous 2D DMA, and stay SBUF-resident for the whole kernel
(~170 KB/partition total).

Per-core dataflow, tokens processed in 4 quarters of 512:
  MM1: ph[il,t] = sum_k w1[k,il].T @ xT[k,t]   (8-chain into PSUM)
  gelu -> hT (bf16)
  MM2: py[t,h] = sum_il hT[il,t].T @ w2[il,h]  (4-chain into PSUM)
  y accumulated across the 8 il-groups in SBUF (fp32), DMA'd out
  per 128-token tile as soon as the last group's add completes.
"""

import sys

sys.path.insert(0, "/opt/trn_rl_repo")

import numpy as np

# --- problem constants (hardcoded; kernel.py must be self-contained) ---
E = 8          # experts == cores
H = 1024       # hidden
I = 4096       # intermediate
N_TOK = 16384  # total tokens
T = N_TOK // E  # tokens per core (capacity)

P = 128
NQ = 4               # token quarters per core
TQ = T // NQ         # tokens per quarter (512)
HB = H // P          # 8
IB = I // P          # 32
GI = 4               # I-tiles per group
NG = IB // GI        # 8 groups

N_CORES = E

_CACHE = {}


def _build():
    import concourse.bacc as bacc
    import concourse.mybir as mybir
    import concourse.tile as tile

    F32 = mybir.dt.float32
    BF16 = mybir.dt.bfloat16
    GELU = mybir.ActivationFunctionType.Gelu_apprx_tanh
    COPY = mybir.ActivationFunctionType.Copy

    nc = bacc.Bacc("TRN2", target_bir_lowering=False, debug=False, num_devices=E)

    # host-packed layouts (see _make_in_maps):
    #   xTp[p, k*2048 + q*512 + t] = x[q*512+t, k*128+p]
    #   w1p[p, i*1024 + k*128 + c] = w1[k*128+p, i*128+c]
    #   w2p[p, il*1024 + h]        = w2[il*128+p, h]
    xTp = nc.dram_tensor("xTp", [P, HB * T], BF16, kind="ExternalInput").ap()
    w1p = nc.dram_tensor("w1p", [P, HB * I], BF16, kind="ExternalInput").ap()
    w2p = nc.dram_tensor("w2p", [P, IB * H], BF16, kind="ExternalInput").ap()
    y = nc.dram_tensor("y", [T, H], F32, kind="ExternalOutput").ap()

    with tile.TileContext(nc) as tc:
        with (
            tc.tile_pool(name="xp", bufs=2) as x_pool,
            tc.tile_pool(name="w1p", bufs=1) as w1_pool,
            tc.tile_pool(name="w2p", bufs=1) as w2_pool,
            tc.tile_pool(name="ysb", bufs=1) as y_pool,
            tc.tile_pool(name="hT", bufs=8) as hT_pool,
            tc.tile_pool(name="warm", bufs=1) as warm_pool,
            tc.tile_pool(name="ph", bufs=6, space="PSUM") as ph_pool,
            tc.tile_pool(name="py", bufs=2, space="PSUM") as py_pool,
        ):
            # PE warmup while first DMAs land (keeps clock ramping).
            wsrc = warm_pool.tile([P, TQ], BF16, tag="warm", name="wsrc")
            nc.gpsimd.memset(wsrc[:], 0.0)
            for _ in range(10):
                pw = ph_pool.tile([P, TQ], F32, tag="ph", name="pw")
                nc.tensor.matmul(pw[:], wsrc[:, :P], wsrc[:], start=True, stop=True)

            w1t = w1_pool.tile([P, HB * I], BF16, tag="w1", name="w1t")
            w2t = w2_pool.tile([P, IB * H], BF16, tag="w2", name="w2t")

            xts = {}

            def load_q(q):
                for k in range(HB):
                    xt = x_pool.tile([P, TQ], BF16, tag=f"x{k}", name=f"x{q}_{k}")
                    nc.sync.dma_start(
                        out=xt[:],
                        in_=xTp[:, k * T + q * TQ : k * T + (q + 1) * TQ],
                    )
                    xts[(q, k)] = xt

            # DMA priority order: w1 group 0, first token quarter, w2 group 0,
            # then the remaining weight chunks.
            nc.sync.dma_start(out=w1t[:, : GI * H], in_=w1p[:, : GI * H])
            load_q(0)
            nc.sync.dma_start(out=w2t[:, : GI * H], in_=w2p[:, : GI * H])
            for g in range(1, NG):
                gs = slice(g * GI * H, (g + 1) * GI * H)
                nc.sync.dma_start(out=w1t[:, gs], in_=w1p[:, gs])
                nc.sync.dma_start(out=w2t[:, gs], in_=w2p[:, gs])

            ysb = {}

            def mm1_block(q, g):
                hTt = []
                for il in range(GI):
                    i = g * GI + il
                    ph = ph_pool.tile([P, TQ], F32, tag="ph", name="ph")
                    for k in range(HB):
                        nc.tensor.matmul(
                            ph[:],
                            w1t[:, i * H + k * P : i * H + (k + 1) * P],
                            xts[(q, k)][:],
                            start=(k == 0),
                            stop=(k == HB - 1),
                        )
                    ht = hT_pool.tile([P, TQ], BF16, tag="ht", name="ht")
                    nc.scalar.activation(ht[:], ph[:], GELU)
                    hTt.append(ht)
                return hTt

            def mm2_block(q, g, hTt):
                for tci in range(TQ // P):
                    for hh in range(2):
                        hs = slice(hh * (H // 2), (hh + 1) * (H // 2))
                        py = py_pool.tile([P, H // 2], F32, tag="py", name="py")
                        for il in range(GI):
                            nc.tensor.matmul(
                                py[:],
                                hTt[il][:, tci * P : (tci + 1) * P],
                                w2t[
                                    :,
                                    g * GI * H + il * H + hh * (H // 2)
                                    : g * GI * H + il * H + (hh + 1) * (H // 2),
                                ],
                                start=(il == 0),
                                stop=(il == GI - 1),
                            )
                        if g == 0:
                            if hh == 0:
                                ysb[tci] = y_pool.tile(
                                    [P, H], F32, tag=f"y{tci}", name=f"y{q}_{tci}"
                                )
                            nc.scalar.activation(ysb[tci][:, hs], py[:], COPY)
                        else:
                            nc.vector.tensor_add(
                                ysb[tci][:, hs], ysb[tci][:, hs], py[:]
                            )
                        if g == NG - 1:
                            # post each half as soon as its last add lands so
                            # the final DMA drain overlaps the last chains;
                            # quarter-width posts in the last token quarter
                            # spread the final transfers across queues
                            t0 = q * TQ + tci * P
                            if q == NQ - 1:
                                for qh in range(2):
                                    qs = slice(
                                        hh * (H // 2) + qh * (H // 4),
                                        hh * (H // 2) + (qh + 1) * (H // 4),
                                    )
                                    nc.sync.dma_start(
                                        out=y[t0 : t0 + P, qs], in_=ysb[tci][:, qs]
                                    )
                            else:
                                nc.sync.dma_start(
                                    out=y[t0 : t0 + P, hs], in_=ysb[tci][:, hs]
                                )

            # software-pipelined by one group: MM1(n+1) issues before MM2(n),
            # so MM2's gelu inputs are a full block ahead of the PE.
            prev = None
            for q in range(NQ):
                if q + 1 < NQ:
                    load_q(q + 1)
                for g in range(NG):
                    hTt = mm1_block(q, g)
                    if prev is not None:
                        mm2_block(*prev)
                    prev = (q, g, hTt)
            mm2_block(*prev)

    nc.compile()
    return nc


def _get_nc():
    if "nc" not in _CACHE:
        _CACHE["nc"] = _build()
    return _CACHE["nc"]


def _row_index(inputs):
    counts = np.asarray(inputs["expert_counts"], dtype=np.int64)
    n = np.asarray(inputs["x_sorted"]).shape[0]
    offsets = np.cumsum(counts)
    # per-token expert id, identical to reference's searchsorted
    eid = np.searchsorted(offsets, np.arange(n), side="right")
    return [np.nonzero(eid == e)[0] for e in range(E)]


def _make_in_maps(inputs):
    import ml_dtypes

    BF = ml_dtypes.bfloat16
    x_sorted = np.ascontiguousarray(inputs["x_sorted"], dtype=np.float32)
    w1 = np.ascontiguousarray(inputs["w1"], dtype=np.float32)
    w2 = np.ascontiguousarray(inputs["w2"], dtype=np.float32)
    in_maps = []
    for e, rows in enumerate(_row_index(inputs)):
        assert len(rows) <= T, f"expert {e} overflows capacity {T}"
        xe = np.zeros((T, H), dtype=np.float32)
        xe[: len(rows)] = x_sorted[rows]
        # xTp[p, k*T + q*TQ + t] = xe[q*TQ+t, k*P+p]
        xTp = (
            xe.reshape(NQ, TQ, HB, P).transpose(3, 2, 0, 1).reshape(P, HB * T)
        )
        # w1p[p, i*H + k*P + c] = w1[e][k*P+p, i*P+c]
        w1p = (
            w1[e].reshape(HB, P, IB, P).transpose(1, 2, 0, 3).reshape(P, HB * I)
        )
        # w2p[p, il*H + h] = w2[e][il*P+p, h]
        w2p = w2[e].reshape(IB, P, H).transpose(1, 0, 2).reshape(P, IB * H)
        in_maps.append(
            {
                "xTp": np.ascontiguousarray(xTp).astype(BF),
                "w1p": np.ascontiguousarray(w1p).astype(BF),
                "w2p": np.ascontiguousarray(w2p).astype(BF),
            }
        )
    return in_maps


def _gather(inputs, results):
    n = np.asarray(inputs["x_sorted"]).shape[0]
    out = np.zeros((n, H), dtype=np.float32)
    for e, rows in enumerate(_row_index(inputs)):
        out[rows] = results[e]["y"][: len(rows)]
    return out


def kernel(x_sorted, w1, w2, expert_counts, local_expert_indices, **_unused):
    from concourse.bass_utils import run_bass_kernel_spmd

    inputs = {
        "x_sorted": x_sorted,
        "w1": w1,
        "w2": w2,
        "expert_counts": expert_counts,
    }
    nc = _get_nc()
    res = run_bass_kernel_spmd(nc, _make_in_maps(inputs), list(range(E))).results
    return _gather(inputs, res)


# revision 15
# speedup vs baseline: 1.0148x; 1.0148x over previous
"""MoE ExpertGroup kernel for Trainium2 (8 NeuronCores, expert-parallel).

Problem: E=8 experts, H=1024, I=4096, N=16384 tokens sorted by expert.
y[t] = gelu_tanh(x[t] @ w1[e(t)]) @ w2[e(t)]

Sharding: expert-parallel - core e holds expert e's weights and processes
expert e's contiguous token block (balanced routing: 2048 tokens/core).

All matmul operands are bf16 (PE streams 512-row matmuls at ~216 ns vs
227 ns for fp32r); PSUM accumulation and the final y stay fp32. Weights
are packed host-side into partition-major layouts so each logical load
is one contiguous 2D DMA, and stay SBUF-resident for the whole kernel
(~170 KB/partition total).

Per-core dataflow, tokens processed in 4 quarters of 512:
  MM1: ph[il,t] = sum_k w1[k,il].T @ xT[k,t]   (8-chain into PSUM)
  gelu -> hT (bf16)
  MM2: py[t,h] = sum_il hT[il,t].T @ w2[il,h]  (4-chain into PSUM)
  y accumulated across the 8 il-groups in SBUF (fp32), DMA'd out
  per 128-token tile as soon as the last group's add completes.
"""

import sys

sys.path.insert(0, "/opt/trn_rl_repo")

import numpy as np

# --- problem constants (hardcoded; kernel.py must be self-contained) ---
E = 8          # experts == cores
H = 1024       # hidden
I = 4096       # intermediate
N_TOK = 16384  # total tokens
T = N_TOK // E  # tokens per core (capacity)

P = 128
NQ = 4               # token quarters per core
TQ = T // NQ         # tokens per quarter (512)
HB = H // P          # 8
IB = I // P          # 32
GI = 4               # I-tiles per group
NG = IB // GI        # 8 groups

N_CORES = E

_CACHE = {}


def _build():
    import concourse.bacc as bacc
    import concourse.mybir as mybir
    import concourse.tile as tile

    F32 = mybir.dt.float32
    BF16 = mybir.dt.bfloat16
    GELU = mybir.ActivationFunctionType.Gelu_apprx_tanh
    COPY = mybir.ActivationFunctionType.Copy

    nc = bacc.Bacc("TRN2", target_bir_lowering=False, debug=False, num_devices=E)

    # host-packed layouts (see _make_in_maps):
    #   xTp[p, k*2048 + q*512 + t] = x[q*512+t, k*128+p]
    #   w1p[p, i*1024 + k*128 + c] = w1[k*128+p, i*128+c]
    #   w2p[p, il*1024 + h]        = w2[il*128+p, h]
    xTp = nc.dram_tensor("xTp", [P, HB * T], BF16, kind="ExternalInput").ap()
    w1p = nc.dram_tensor("w1p", [P, HB * I], BF16, kind="ExternalInput").ap()
    w2p = nc.dram_tensor("w2p", [P, IB * H], BF16, kind="ExternalInput").ap()
    y = nc.dram_tensor("y", [T, H], F32, kind="ExternalOutput").ap()

    with tile.TileContext(nc) as tc:
        with (
            tc.tile_pool(name="xp", bufs=2) as x_pool,
            tc.tile_pool(name="w1p", bufs=1) as w1_pool,
            tc.tile_pool(name="w2p", bufs=1) as w2_pool,
            tc.tile_pool(name="ysb", bufs=1) as y_pool,
            tc.tile_pool(name="hT", bufs=8) as hT_pool,
            tc.tile_pool(name="warm", bufs=1) as warm_pool,
            tc.tile_pool(name="ph", bufs=6, space="PSUM") as ph_pool,
            tc.tile_pool(name="py", bufs=2, space="PSUM") as py_pool,
        ):
            # PE warmup while first DMAs land (keeps clock ramping).
            wsrc = warm_pool.tile([P, TQ], BF16, tag="warm", name="wsrc")
            nc.gpsimd.memset(wsrc[:], 0.0)
            for _ in range(10):
                pw = ph_pool.tile([P, TQ], F32, tag="ph", name="pw")
                nc.tensor.matmul(pw[:], wsrc[:, :P], wsrc[:], start=True, stop=True)

            w1t = w1_pool.tile([P, HB * I], BF16, tag="w1", name="w1t")
            w2t = w2_pool.tile([P, IB * H], BF16, tag="w2", name="w2t")

            xts = {}

            def load_q(q):
                for k in range(HB):
                    xt = x_pool.tile([P, TQ], BF16, tag=f"x{k}", name=f"x{q}_{k}")
                    nc.sync.dma_start(
                        out=xt[:],
                        in_=xTp[:, k * T + q * TQ : k * T + (q + 1) * TQ],
                    )
                    xts[(q, k)] = xt

            # DMA priority order: w1 group 0, first token quarter, w2 group 0,
            # then the remaining weight chunks.
            nc.sync.dma_start(out=w1t[:, : GI * H], in_=w1p[:, : GI * H])
            load_q(0)
            nc.sync.dma_start(out=w2t[:, : GI * H], in_=w2p[:, : GI * H])
            for g in range(1, NG):
                gs = slice(g * GI * H, (g + 1) * GI * H)
                nc.sync.dma_start(out=w1t[:, gs], in_=w1p[:, gs])
                nc.sync.dma_start(out=w2t[:, gs], in_=w2p[:, gs])

            ysb = {}

            def mm1_block(q, g):
                hTt = []
                for il in range(GI):
                    i = g * GI + il
                    ph = ph_pool.tile([P, TQ], F32, tag="ph", name="ph")
                    for k in range(HB):
                        nc.tensor.matmul(
                            ph[:],
                            w1t[:, i * H + k * P : i * H + (k + 1) * P],
                            xts[(q, k)][:],
                            start=(k == 0),
                            stop=(k == HB - 1),
                        )
                    ht = hT_pool.tile([P, TQ], BF16, tag="ht", name="ht")
                    nc.scalar.activation(ht[:], ph[:], GELU)
                    hTt.append(ht)
                return hTt

            def mm2_block(q, g, hTt):
                for tci in range(TQ // P):
                    for hh in range(2):
                        hs = slice(hh * (H // 2), (hh + 1) * (H // 2))
                        py = py_pool.tile([P, H // 2], F32, tag="py", name="py")
                        for il in range(GI):
                            nc.tensor.matmul(
                                py[:],
                                hTt[il][:, tci * P : (tci + 1) * P],
                                w2t[
                                    :,
                                    g * GI * H + il * H + hh * (H // 2)
                                    : g * GI * H + il * H + (hh + 1) * (H // 2),
                                ],
                                start=(il == 0),
                                stop=(il == GI - 1),
                            )
                        if g == 0:
                            if hh == 0:
                                ysb[tci] = y_pool.tile(
                                    [P, H], F32, tag=f"y{tci}", name=f"y{q}_{tci}"
                                )
                            nc.scalar.activation(ysb[tci][:, hs], py[:], COPY)
                        else:
                            nc.vector.tensor_add(
                                ysb[tci][:, hs], ysb[tci][:, hs], py[:]
                            )
                        if g == NG - 1:
                            # post each half as soon as its last add lands so
                            # the final DMA drain overlaps the last chains
                            t0 = q * TQ + tci * P
                            nc.sync.dma_start(
                                out=y[t0 : t0 + P, hs], in_=ysb[tci][:, hs]
                            )

            # software-pipelined by one group: MM1(n+1) issues before MM2(n),
            # so MM2's gelu inputs are a full block ahead of the PE.
            prev = None
            for q in range(NQ):
                if q + 1 < NQ:
                    load_q(q + 1)
                for g in range(NG):
                    hTt = mm1_block(q, g)
                    if prev is not None:
                        mm2_block(*prev)
                    prev = (q, g, hTt)
            mm2_block(*prev)

    nc.compile()
    return nc


def _get_nc():
    if "nc" not in _CACHE:
        _CACHE["nc"] = _build()
    return _CACHE["nc"]


def _row_index(inputs):
    counts = np.asarray(inputs["expert_counts"], dtype=np.int64)
    n = np.asarray(inputs["x_sorted"]).shape[0]
    offsets = np.cumsum(counts)
    # per-token expert id, identical to reference's searchsorted
    eid = np.searchsorted(offsets, np.arange(n), side="right")
    return [np.nonzero(eid == e)[0] for e in range(E)]


def _make_in_maps(inputs):
    import ml_dtypes

    BF = ml_dtypes.bfloat16
    x_sorted = np.ascontiguousarray(inputs["x_sorted"], dtype=np.float32)
    w1 = np.ascontiguousarray(inputs["w1"], dtype=np.float32)
    w2 = np.ascontiguousarray(inputs["w2"], dtype=np.float32)
    in_maps = []
    for e, rows in enumerate(_row_index(inputs)):
        assert len(rows) <= T, f"expert {e} overflows capacity {T}"
        xe = np.zeros((T, H), dtype=np.float32)
        xe[: len(rows)] = x_sorted[rows]
        # xTp[p, k*T + q*TQ + t] = xe[q*TQ+t, k*P+p]
        xTp = (
            xe.reshape(NQ, TQ, HB, P).transpose(3, 2, 0, 1).reshape(P, HB * T)
        )
        # w1p[p, i*H + k*P + c] = w1[e][k*P+p, i*P+c]
        w1p = (
            w1[e].reshape(HB, P, IB, P).transpose(1, 2, 0, 3).reshape(P, HB * I)
        )
        # w2p[p, il*H + h] = w2[e][il*P+p, h]
        w2p = w2[e].reshape(IB, P, H).transpose(1, 0, 2).reshape(P, IB * H)
        in_maps.append(
            {
                "xTp": np.ascontiguousarray(xTp).astype(BF),
                "w1p": np.ascontiguousarray(w1p).astype(BF),
                "w2p": np.ascontiguousarray(w2p).astype(BF),
            }
        )
    return in_maps


def _gather(inputs, results):
    n = np.asarray(inputs["x_sorted"]).shape[0]
    out = np.zeros((n, H), dtype=np.float32)
    for e, rows in enumerate(_row_index(inputs)):
        out[rows] = results[e]["y"][: len(rows)]
    return out


def kernel(x_sorted, w1, w2, expert_counts, local_expert_indices, **_unused):
    from concourse.bass_utils import run_bass_kernel_spmd

    inputs = {
        "x_sorted": x_sorted,
        "w1": w1,
        "w2": w2,
        "expert_counts": expert_counts,
    }
    nc = _get_nc()
    res = run_bass_kernel_spmd(nc, _make_in_maps(inputs), list(range(E))).results
    return _gather(inputs, res)
